# revision 3
# baseline (speedup 1.0000x reference)
"""Trainium2 Bass kernel for nn_DenseProduct (num_factors=2).

Computes, for input x of shape (128, 16, 64, 32) f32:
    out[s, d, b, i*32+j] = x[2s, d, b, i] + x[2s+1, d, b, j]
with output shape (64, 16, 64, 1024) f32.

Sharding: scope axis (dim 0) across 8 NeuronCores — core c gets input
scopes [16c, 16c+16) and produces output scopes [8c, 8c+8), a contiguous
33.5 MB slice of the output per core.

Per-core layout: SBUF partition p = d*8 + b_hi (d in [0,16), b_hi in [0,8),
b = 8*b_hi + b_lo). Input DMA reads contiguous 1 KB runs; the output DMA
writes contiguous regions of the 4 MB per-scope block (32 KB/partition).

The kernel is DMA-write-bound: the 16-engine DMA cluster sustains
~450-460 GB/s/core, so the 33.5 MB output needs ~75 us on the wire. The
adds are therefore split across two engines so production always outruns
the drain (DVE: 1 elem/cycle @0.96 GHz = 123 GB/s-of-output per partition
row set; Pool: ~0.42 eff @1.2 GHz = ~64 GB/s): DVE takes bl-blocks 0-4 of
each scope, Pool takes bl-blocks 5-7. DVE-piece outputs stream on the
sync HWDGE ring and Pool-piece outputs on the scalar ring, so each ring's
program order matches its producer's completion order (no cross-producer
head-of-line blocking) and each ring's per-DMA completion boundary hides
under the other ring's stream. Input DMAs are issued on both rings in
parallel up front; scope 0 ramps up with small pieces so the output
stream saturates as early as possible.
"""

import numpy as np

_S_IN = 128        # total input scopes
_NF = 2            # num_factors (hardcoded)
_S_OUT = _S_IN // _NF
_D = 16
_B = 64
_N = 32
_N_CORES = 8
_SIN_LOC = _S_IN // _N_CORES   # 16 input scopes per core
_S_LOC = _S_OUT // _N_CORES    # 8 output scopes per core
_P = 128
_BH = 8
_BL = 8
_FREE_IN = _BL * _N            # 256
_FREE_OUT = _BL * _N * _N      # 8192
_BL_V = 5                      # bl-blocks 0..4 -> DVE
# bl-blocks 5..7 -> Pool

_CACHE = {}
LAST_RESULTS = None  # BassKernelResults of the most recent run (for profiling)


def _build_bass():
    import concourse.bacc as bacc
    import concourse.mybir as mybir
    from concourse.tile import TileContext

    nc = bacc.Bacc("TRN2", target_bir_lowering=False, debug=False,
                   num_devices=_N_CORES)
    x = nc.dram_tensor("x", [_SIN_LOC, _D, _B, _N], mybir.dt.float32,
                       kind="ExternalInput").ap()
    out = nc.dram_tensor("out", [_S_LOC, _D, _B, _N * _N], mybir.dt.float32,
                         kind="ExternalOutput").ap()

    with TileContext(nc) as tc:
        with tc.tile_pool(name="inp", bufs=_S_LOC) as in_pool, \
             tc.tile_pool(name="head", bufs=1) as head_pool, \
             tc.tile_pool(name="outp", bufs=4) as out_pool:
            # x[s_in, d, 8*bh+bl, n] -> partition (d, bh), free (s_in, bl, n)
            xr = x.rearrange("s d (bh bl) n -> (d bh) s (bl n)", bh=_BH)
            # tiny head tile: bl=0 strip of both factors of scope 0, so the
            # very first compute piece (and with it the output DMA stream)
            # starts well before the full scope-0 input lands
            ht = head_pool.tile([_P, 2 * _N], mybir.dt.float32)
            nc.sync.dma_start(out=ht[:, :].rearrange("p (s f) -> p s f", s=2),
                              in_=xr[:, 0:2, 0:_N])
            in_tiles = []
            for s in range(_S_LOC):
                # both factors (s_in = 2s, 2s+1) in one DMA -> one wait sem;
                # even scopes issue on the sync ring, odd on the scalar ring
                # so all input DMAs are in flight within ~3 us of preamble end
                t = in_pool.tile([_P, 2 * _FREE_IN], mybir.dt.float32)
                src = xr[:, 2 * s:2 * s + 2]  # (128, 2, 256), s-stride 32768
                dst = t[:, :].rearrange("p (s f) -> p s f", s=2)
                eng = nc.sync if s % 2 == 0 else nc.scalar
                eng.dma_start(out=dst, in_=src)
                in_tiles.append(t)

            def emit(add_eng, dma_eng, s, ot, bl0, w, i0, wi, src_t=None):
                """One add piece (w bl-blocks, wi i-values from i0) and its
                output DMA. src offset pair: factor a at 0, factor b at
                _FREE_IN."""
                if src_t is None:
                    src_t = in_tiles[s]
                    off_a, off_b = 0, _FREE_IN
                else:
                    off_a, off_b = 0, _N  # head tile: [a_strip, b_strip]
                a = src_t[:, off_a + bl0 * (_N if src_t is not ht else 0) + i0:
                          off_a + bl0 * (_N if src_t is not ht else 0) + i0
                          + (w - 1) * _N + wi]
                # (the head tile only holds bl0=0, so bl offset is 0 there)
                a = a.rearrange("p (bl i) -> p bl i", bl=w)
                b = src_t[:, off_b + bl0 * (_N if src_t is not ht else 0):
                          off_b + bl0 * (_N if src_t is not ht else 0) + w * _N]
                b = b.rearrange("p (bl j) -> p bl j", bl=w)
                a4 = a.unsqueeze(3).broadcast_to([_P, w, wi, _N])
                b4 = b.unsqueeze(2).broadcast_to([_P, w, wi, _N])
                f0 = bl0 * _N * _N + i0 * _N
                sz = w * wi * _N
                osl = ot[:, f0:f0 + sz]
                o4 = osl.rearrange("p (bl i j) -> p bl i j", bl=w, i=wi)
                add_eng.tensor_add(o4, a4, b4)
                dst = out[s].rearrange("d (bh bl) f -> (d bh) (bl f)", bh=_BH)
                dma_eng.dma_start(out=dst[:, f0:f0 + sz], in_=osl)

            for s in range(_S_LOC):
                ot = out_pool.tile([_P, _FREE_OUT], mybir.dt.float32)
                if s == 0:
                    # ramp-up: tiny first pieces from the head tile, then the
                    # rest of the DVE region from the full scope-0 input
                    emit(nc.vector, nc.sync, s, ot, 0, 1, 0, 16, src_t=ht)
                    emit(nc.vector, nc.scalar, s, ot, 0, 1, 16, 16, src_t=ht)
                    emit(nc.vector, nc.sync, s, ot, 1, 1, 0, _N)
                    emit(nc.gpsimd, nc.scalar, s, ot, 5, 1, 0, _N)
                    emit(nc.vector, nc.sync, s, ot, 2, 2, 0, _N)
                    emit(nc.gpsimd, nc.scalar, s, ot, 6, 2, 0, _N)
                    emit(nc.vector, nc.sync, s, ot, 4, 1, 0, _N)
                elif s == 1:
                    emit(nc.vector, nc.sync, s, ot, 0, 2, 0, _N)
                    emit(nc.gpsimd, nc.scalar, s, ot, 5, 3, 0, _N)
                    emit(nc.vector, nc.sync, s, ot, 2, 3, 0, _N)
                else:
                    # steady state: one DVE piece (bl 0-4) on the sync ring,
                    # one Pool piece (bl 5-7) on the scalar ring
                    emit(nc.vector, nc.sync, s, ot, 0, _BL_V, 0, _N)
                    emit(nc.gpsimd, nc.scalar, s, ot, _BL_V, _BL - _BL_V, 0, _N)
    nc.compile()
    return nc


def kernel(x, num_factors):
    global LAST_RESULTS
    from concourse.bass_utils import run_bass_kernel_spmd

    x = np.asarray(x)
    assert x.shape == (_S_IN, _D, _B, _N), x.shape
    assert int(num_factors) == _NF, num_factors
    x = x.astype(np.float32, copy=False)

    if "nc" not in _CACHE:
        _CACHE["nc"] = _build_bass()
    nc = _CACHE["nc"]

    in_maps = [
        {"x": np.ascontiguousarray(x[c * _SIN_LOC:(c + 1) * _SIN_LOC])}
        for c in range(_N_CORES)
    ]
    res = run_bass_kernel_spmd(nc, in_maps, core_ids=list(range(_N_CORES)))
    LAST_RESULTS = res
    out = np.concatenate([res.results[c]["out"] for c in range(_N_CORES)], axis=0)
    return out.reshape(_S_OUT, _D, _B, _N ** _NF)


# revision 7
# speedup vs baseline: 1.1403x; 1.1403x over previous
"""Trainium2 Bass kernel for nn_DenseProduct (num_factors=2).

Computes, for input x of shape (128, 16, 64, 32) f32:
    out[s, d, b, i*32+j] = x[2s, d, b, i] + x[2s+1, d, b, j]
with output shape (64, 16, 64, 1024) f32.

Sharding: scope axis (dim 0) across 8 NeuronCores — core c gets input
scopes [16c, 16c+16) and produces output scopes [8c, 8c+8), a contiguous
33.5 MB slice of the output per core.

Per-core layout: SBUF partition p = d*8 + b_hi (d in [0,16), b_hi in [0,8),
b = 8*b_hi + b_lo). Input DMA reads contiguous 1 KB runs; the output DMA
writes contiguous regions of the 4 MB per-scope block (32 KB/partition).

Roofline: the 16-SDMA cluster sustains ~450 GB/s/core, so the 35.7 MB of
traffic (2.1 in + 33.5 out) needs ~79 us on the wire — the kernel is
DMA-bound. DVE produces the adds at 0.96 elem/ns (fp32 1x; GpSimd shares
DVE's SBUF port with an exclusive lock, so a second adder engine gains
nothing). The schedule therefore aims everything at keeping the SDMA
cluster saturated:
  - head strip DMA + scope-0/1 input issued immediately on both HWDGE
    rings (sync + scalar) so the first add starts ~2.3 us after the
    preamble;
  - scope 0 ramps with doubling piece sizes so output DMAs enter the
    queues as early as possible;
  - remaining input DMAs are staggered between output issues — their
    packets fill the gaps while DVE production (480 GB/s) only slowly
    outruns the drain (450 GB/s);
  - output DMAs strictly alternate rings so each DMA's ~1 us completion
    boundary hides under the other ring's stream.
"""

import numpy as np

_S_IN = 128        # total input scopes
_NF = 2            # num_factors (hardcoded)
_S_OUT = _S_IN // _NF
_D = 16
_B = 64
_N = 32
_N_CORES = 8
_SIN_LOC = _S_IN // _N_CORES   # 16 input scopes per core
_S_LOC = _S_OUT // _N_CORES    # 8 output scopes per core
_P = 128
_BH = 8
_BL = 8
_FREE_IN = _BL * _N            # 256
_FREE_OUT = _BL * _N * _N      # 8192

_CACHE = {}
LAST_RESULTS = None  # BassKernelResults of the most recent run (for profiling)


def _build_bass():
    import concourse.bacc as bacc
    import concourse.mybir as mybir
    from concourse.tile import TileContext

    nc = bacc.Bacc("TRN2", target_bir_lowering=False, debug=False,
                   num_devices=_N_CORES)
    x = nc.dram_tensor("x", [_SIN_LOC, _D, _B, _N], mybir.dt.float32,
                       kind="ExternalInput").ap()
    out = nc.dram_tensor("out", [_S_LOC, _D, _B, _N * _N], mybir.dt.float32,
                         kind="ExternalOutput").ap()

    with TileContext(nc) as tc:
        with tc.tile_pool(name="inp", bufs=_S_LOC) as in_pool, \
             tc.tile_pool(name="head", bufs=1) as head_pool, \
             tc.tile_pool(name="outp", bufs=4) as out_pool:
            # x[s_in, d, 8*bh+bl, n] -> partition (d, bh), free (s_in, bl, n)
            xr = x.rearrange("s d (bh bl) n -> (d bh) s (bl n)", bh=_BH)
            # tiny head tile: bl=0 strip of both factors of scope 0, so the
            # very first compute piece (and with it the output DMA stream)
            # starts ~2.3us after the preamble, before scope-0 input lands
            ht = head_pool.tile([_P, 2 * _N], mybir.dt.float32)
            nc.sync.dma_start(out=ht[:, :].rearrange("p (s f) -> p s f", s=2),
                              in_=xr[:, 0:2, 0:_N])

            in_tiles = [in_pool.tile([_P, 2 * _FREE_IN], mybir.dt.float32,
                                     name="it") for s in range(_S_LOC)]

            def load_input(s, eng):
                # both factors (s_in = 2s, 2s+1) in one DMA -> one wait sem
                src = xr[:, 2 * s:2 * s + 2]  # (128, 2, 256), s-stride 32768
                dst = in_tiles[s][:, :].rearrange("p (s f) -> p s f", s=2)
                eng.dma_start(out=dst, in_=src)

            # scope 0 + 1 inputs up front (parallel on both rings); the rest
            # are interleaved with output issues below so their packets fill
            # ramp-up gaps in the output stream
            load_input(0, nc.sync)
            load_input(1, nc.scalar)

            state = {"ndma": 0}

            def emit_add(s, ot, bl0, w, i0, wi, use_head=False):
                """One DVE add piece (w bl-blocks, wi i-values from i0) and
                its output DMA (rings strictly alternate)."""
                if use_head:
                    src_t, off_a, off_b = ht, 0, _N
                else:
                    src_t = in_tiles[s]
                    off_a, off_b = bl0 * _N, _FREE_IN + bl0 * _N
                a = src_t[:, off_a + i0:off_a + i0 + (w - 1) * _N + wi] \
                    .rearrange("p (bl i) -> p bl i", bl=w)
                b = src_t[:, off_b:off_b + w * _N] \
                    .rearrange("p (bl j) -> p bl j", bl=w)
                a4 = a.unsqueeze(3).broadcast_to([_P, w, wi, _N])
                b4 = b.unsqueeze(2).broadcast_to([_P, w, wi, _N])
                f0 = bl0 * _N * _N + i0 * _N
                sz = w * wi * _N
                osl = ot[:, f0:f0 + sz]
                o4 = osl.rearrange("p (bl i j) -> p bl i j", bl=w, i=wi)
                nc.vector.tensor_add(o4, a4, b4)
                dst = out[s].rearrange("d (bh bl) f -> (d bh) (bl f)", bh=_BH)
                eng = nc.sync if state["ndma"] % 2 == 0 else nc.scalar
                eng.dma_start(out=dst[:, f0:f0 + sz], in_=osl)
                state["ndma"] += 1

            # remaining inputs: scope s's input is issued alongside scope
            # s-2's output pieces (well before scope s's adds need it)
            pending_inputs = list(range(2, _S_LOC))

            for s in range(_S_LOC):
                ot = out_pool.tile([_P, _FREE_OUT], mybir.dt.float32)
                if s == 0:
                    pieces = [(0, 1, 0, 16, True), (0, 1, 16, 16, True),
                              (1, 1, 0, _N, False), (2, 2, 0, _N, False),
                              (4, 4, 0, _N, False)]
                elif s == 1:
                    pieces = [(0, 4, 0, _N, False), (4, 4, 0, _N, False)]
                else:
                    pieces = [(0, _BL, 0, _N, False)]
                for k, (bl0, w, i0, wi, uh) in enumerate(pieces):
                    emit_add(s, ot, bl0, w, i0, wi, use_head=uh)
                    if k == len(pieces) - 1 and pending_inputs:
                        # stagger the next input load after this scope's
                        # last output issue, on the ring that issues next
                        eng = nc.sync if state["ndma"] % 2 == 0 else nc.scalar
                        load_input(pending_inputs.pop(0), eng)
    nc.compile()
    return nc


def kernel(x, num_factors):
    global LAST_RESULTS
    from concourse.bass_utils import run_bass_kernel_spmd

    x = np.asarray(x)
    assert x.shape == (_S_IN, _D, _B, _N), x.shape
    assert int(num_factors) == _NF, num_factors
    x = x.astype(np.float32, copy=False)

    if "nc" not in _CACHE:
        _CACHE["nc"] = _build_bass()
    nc = _CACHE["nc"]

    in_maps = [
        {"x": np.ascontiguousarray(x[c * _SIN_LOC:(c + 1) * _SIN_LOC])}
        for c in range(_N_CORES)
    ]
    res = run_bass_kernel_spmd(nc, in_maps, core_ids=list(range(_N_CORES)))
    LAST_RESULTS = res
    out = np.concatenate([res.results[c]["out"] for c in range(_N_CORES)], axis=0)
    return out.reshape(_S_OUT, _D, _B, _N ** _NF)


# revision 9
# speedup vs baseline: 1.1515x; 1.0098x over previous
"""Trainium2 Bass kernel for nn_DenseProduct (num_factors=2).

Computes, for input x of shape (128, 16, 64, 32) f32:
    out[s, d, b, i*32+j] = x[2s, d, b, i] + x[2s+1, d, b, j]
with output shape (64, 16, 64, 1024) f32.

Sharding: scope axis (dim 0) across 8 NeuronCores — core c gets input
scopes [16c, 16c+16) and produces output scopes [8c, 8c+8), a contiguous
33.5 MB slice of the output per core.

Per-core layout: SBUF partition p = d*8 + b_hi (d in [0,16), b_hi in [0,8),
b = 8*b_hi + b_lo). Input DMA reads contiguous 1 KB runs; the output DMA
writes contiguous regions of the 4 MB per-scope block (32 KB/partition).

Roofline: the 16-SDMA cluster sustains ~450 GB/s/core, so the 35.7 MB of
traffic (2.1 in + 33.5 out) needs ~79 us on the wire — the kernel is
DMA-bound. DVE produces the adds at 0.96 elem/ns (fp32 1x; GpSimd shares
DVE's SBUF port with an exclusive lock, so a second adder engine gains
nothing). The schedule therefore aims everything at keeping the SDMA
cluster saturated:
  - head strip DMA + scope-0/1 input issued immediately on both HWDGE
    rings (sync + scalar) so the first add starts ~2.3 us after the
    preamble;
  - scope 0 ramps with doubling piece sizes so output DMAs enter the
    queues as early as possible;
  - remaining input DMAs are staggered between output issues — their
    packets fill the gaps while DVE production (480 GB/s) only slowly
    outruns the drain (450 GB/s);
  - output DMAs strictly alternate rings so each DMA's ~1 us completion
    boundary hides under the other ring's stream.
"""

import numpy as np

_S_IN = 128        # total input scopes
_NF = 2            # num_factors (hardcoded)
_S_OUT = _S_IN // _NF
_D = 16
_B = 64
_N = 32
_N_CORES = 8
_SIN_LOC = _S_IN // _N_CORES   # 16 input scopes per core
_S_LOC = _S_OUT // _N_CORES    # 8 output scopes per core
_P = 128
_BH = 8
_BL = 8
_FREE_IN = _BL * _N            # 256
_FREE_OUT = _BL * _N * _N      # 8192

_CACHE = {}
LAST_RESULTS = None  # BassKernelResults of the most recent run (for profiling)


def _build_bass():
    import concourse.bacc as bacc
    import concourse.mybir as mybir
    from concourse.tile import TileContext

    nc = bacc.Bacc("TRN2", target_bir_lowering=False, debug=False,
                   num_devices=_N_CORES)
    x = nc.dram_tensor("x", [_SIN_LOC, _D, _B, _N], mybir.dt.float32,
                       kind="ExternalInput").ap()
    out = nc.dram_tensor("out", [_S_LOC, _D, _B, _N * _N], mybir.dt.float32,
                         kind="ExternalOutput").ap()

    with TileContext(nc) as tc:
        with tc.tile_pool(name="inp", bufs=_S_LOC) as in_pool, \
             tc.tile_pool(name="head", bufs=1) as head_pool, \
             tc.tile_pool(name="outp", bufs=4) as out_pool:
            # x[s_in, d, 8*bh+bl, n] -> partition (d, bh), free (s_in, bl, n)
            xr = x.rearrange("s d (bh bl) n -> (d bh) s (bl n)", bh=_BH)
            # tiny head tile: bl=0 strip of both factors of scope 0, so the
            # very first compute piece (and with it the output DMA stream)
            # starts ~2.3us after the preamble, before scope-0 input lands
            ht = head_pool.tile([_P, 2 * _N], mybir.dt.float32)
            nc.sync.dma_start(out=ht[:, :].rearrange("p (s f) -> p s f", s=2),
                              in_=xr[:, 0:2, 0:_N])

            in_tiles = [in_pool.tile([_P, 2 * _FREE_IN], mybir.dt.float32,
                                     name="it") for s in range(_S_LOC)]

            def load_input(s, eng):
                # both factors (s_in = 2s, 2s+1) in one DMA -> one wait sem
                src = xr[:, 2 * s:2 * s + 2]  # (128, 2, 256), s-stride 32768
                dst = in_tiles[s][:, :].rearrange("p (s f) -> p s f", s=2)
                eng.dma_start(out=dst, in_=src)

            # scope 0-3 inputs up front (parallel on both rings); the rest
            # are interleaved with output issues below so their packets fill
            # ramp-up gaps in the output stream
            load_input(0, nc.sync)
            load_input(1, nc.scalar)
            load_input(2, nc.sync)
            load_input(3, nc.scalar)

            state = {"ndma": 0}

            def emit_add(s, ot, bl0, w, i0, wi, use_head=False):
                """One DVE add piece (w bl-blocks, wi i-values from i0) and
                its output DMA (rings strictly alternate)."""
                if use_head:
                    src_t, off_a, off_b = ht, 0, _N
                else:
                    src_t = in_tiles[s]
                    off_a, off_b = bl0 * _N, _FREE_IN + bl0 * _N
                a = src_t[:, off_a + i0:off_a + i0 + (w - 1) * _N + wi] \
                    .rearrange("p (bl i) -> p bl i", bl=w)
                b = src_t[:, off_b:off_b + w * _N] \
                    .rearrange("p (bl j) -> p bl j", bl=w)
                a4 = a.unsqueeze(3).broadcast_to([_P, w, wi, _N])
                b4 = b.unsqueeze(2).broadcast_to([_P, w, wi, _N])
                f0 = bl0 * _N * _N + i0 * _N
                sz = w * wi * _N
                osl = ot[:, f0:f0 + sz]
                o4 = osl.rearrange("p (bl i j) -> p bl i j", bl=w, i=wi)
                nc.vector.tensor_add(o4, a4, b4)
                dst = out[s].rearrange("d (bh bl) f -> (d bh) (bl f)", bh=_BH)
                eng = nc.sync if state["ndma"] % 2 == 0 else nc.scalar
                eng.dma_start(out=dst[:, f0:f0 + sz], in_=osl)
                state["ndma"] += 1

            # remaining inputs: scope s's input is issued alongside scope
            # s-4's output pieces (two full scope periods before needed)
            pending_inputs = list(range(4, _S_LOC))

            for s in range(_S_LOC):
                ot = out_pool.tile([_P, _FREE_OUT], mybir.dt.float32)
                if s == 0:
                    pieces = [(0, 1, 0, 16, True), (0, 1, 16, 16, True),
                              (1, 1, 0, _N, False), (2, 2, 0, _N, False),
                              (4, 4, 0, _N, False)]
                elif s == 1:
                    pieces = [(0, 4, 0, _N, False), (4, 4, 0, _N, False)]
                elif s == _S_LOC - 1:
                    # last scope: quarter pieces so the final drain is small
                    pieces = [(0, 2, 0, _N, False), (2, 2, 0, _N, False),
                              (4, 2, 0, _N, False), (6, 2, 0, _N, False)]
                else:
                    # steady state: two half pieces, DMAs on opposite rings,
                    # so each half starts draining at the half-add mark and
                    # the SDMA cluster never sits empty between scopes
                    pieces = [(0, 4, 0, _N, False), (4, 4, 0, _N, False)]
                for k, (bl0, w, i0, wi, uh) in enumerate(pieces):
                    emit_add(s, ot, bl0, w, i0, wi, use_head=uh)
                    if k == len(pieces) - 1 and pending_inputs:
                        # stagger the next input load after this scope's
                        # last output issue, on the ring that issues next
                        eng = nc.sync if state["ndma"] % 2 == 0 else nc.scalar
                        load_input(pending_inputs.pop(0), eng)
    nc.compile()
    return nc


def kernel(x, num_factors):
    global LAST_RESULTS
    from concourse.bass_utils import run_bass_kernel_spmd

    x = np.asarray(x)
    assert x.shape == (_S_IN, _D, _B, _N), x.shape
    assert int(num_factors) == _NF, num_factors
    x = x.astype(np.float32, copy=False)

    if "nc" not in _CACHE:
        _CACHE["nc"] = _build_bass()
    nc = _CACHE["nc"]

    in_maps = [
        {"x": np.ascontiguousarray(x[c * _SIN_LOC:(c + 1) * _SIN_LOC])}
        for c in range(_N_CORES)
    ]
    res = run_bass_kernel_spmd(nc, in_maps, core_ids=list(range(_N_CORES)))
    LAST_RESULTS = res
    out = np.concatenate([res.results[c]["out"] for c in range(_N_CORES)], axis=0)
    return out.reshape(_S_OUT, _D, _B, _N ** _NF)
